# revision 1
# baseline (speedup 1.0000x reference)
"""Trainium2 Bass kernel for 2D DWT low-pass (db2): out = mh @ x @ mht per (b,c).

Shapes (hardcoded): input [8, 64, 512, 512] f32, matrix_h [256, 512],
matrix_h_t [512, 256], output [8, 64, 256, 256] f32.

Sharding: data-parallel over the batch dim — core b processes input[b]
(64 images of 512x512); the small filter matrix is replicated.

Math (per image, H == W so matrix_h_t == matrix_h.T):
  stage 1: tmpT[w, p] = sum_h x[h, w] * mht[h, p]        (= (mh @ x).T)
  stage 2: out[p, q]  = sum_w tmpT[w, p] * mht[w, q]     (= mh @ x @ mht)
Both stages are PE matmuls out = lhsT.T @ rhs with the per-image tensor as
the stationary lhsT and the replicated mht as the moving rhs, so no
transposes are ever materialized.

mht is banded: rows 128k..128k+127 are nonzero only in columns
[max(0,64k-1), 64k+64]. Window modes restrict rhs/psum to that band,
cutting PE work ~4x (exact fp32 arithmetic, just skipping zeros).
"""

import os
import sys

sys.path.insert(0, "/opt/trn_rl_repo")

import numpy as np

import concourse.bass as bass
import concourse.tile as tile
from concourse import bacc, mybir
from concourse.bass_utils import run_bass_kernel_spmd

N_CORES = 8
C = 64          # images per core (channel dim; batch is the shard dim)
H = W = 512
P = 128         # SBUF partitions
KH = H // P     # 4 contraction chunks for stage 1
KW = W // P     # 4 contraction chunks for stage 2
NQ = 256        # output columns per matmul (full width)

F32 = mybir.dt.float32
F32R = mybir.dt.float32r

# mode: 'f32'  dense fp32 (exact, 4 cyc/row)
#       'f32w' windowed fp32 (exact, skips the zero band, 4 cyc/row on ~66 cols)
#       'f32r' dense float32r (1 cyc/row at N=256; HW numerics to be validated)
MODE = os.environ.get("DWT_MODE", "f32w")


def _window(k: int) -> tuple[int, int]:
    """Nonzero column range [lo, hi) of mht rows [128k, 128k+128)."""
    lo = max(0, 64 * k - 1)
    hi = min(NQ, 64 * k + 65)
    return lo, hi


def _emit(nc, tc, x_d, mht_d, out_d, mode):
    mm_dt = F32R if mode == "f32r" else F32
    windowed = mode == "f32w"

    def mmcast(ap):
        return ap.bitcast(mm_dt) if mm_dt is not F32 else ap

    with (
        tc.tile_pool(name="consts", bufs=1) as cpool,
        tc.tile_pool(name="img", bufs=3) as img_pool,
        tc.tile_pool(name="tmpT", bufs=2) as tmp_pool,
        tc.tile_pool(name="outp", bufs=3) as out_pool,
        tc.tile_pool(name="ps1", bufs=4, space=bass.MemorySpace.PSUM) as ps1_pool,
        tc.tile_pool(name="ps2", bufs=2, space=bass.MemorySpace.PSUM) as ps2_pool,
    ):
        # mht packed [128, KH, 256]: chunk k = mht[128k:128k+128, :]
        mht_t = cpool.tile([P, KH, NQ], F32)
        nc.sync.dma_start(mht_t[:], mht_d.rearrange("(k p) q -> p k q", p=P))

        for i in range(C):
            img_t = img_pool.tile([P, KH, W], F32, tag="img")
            nc.sync.dma_start(img_t[:], x_d[i].rearrange("(k p) w -> p k w", p=P))

            # stage 1: tmpT[w, p], w-tiles m = 0..3, two per PSUM bank
            tmpT_t = tmp_pool.tile([P, KW, NQ], F32, tag="tmpT")
            for half in range(2):
                ps = ps1_pool.tile([P, 2, NQ], F32, tag="ps1")
                for ml in range(2):
                    m = 2 * half + ml
                    for k in range(KH):
                        lhsT = img_t[:, k, m * P:(m + 1) * P]
                        if windowed:
                            lo, hi = _window(k)
                            rhs = mht_t[:, k, lo:hi]
                            dst = ps[:, ml, lo:hi]
                        else:
                            rhs = mht_t[:, k, :]
                            dst = ps[:, ml, :]
                        nc.tensor.matmul(
                            dst, mmcast(lhsT), mmcast(rhs),
                            start=(k == 0), stop=(k == KH - 1),
                        )
                nc.vector.tensor_copy(tmpT_t[:, 2 * half:2 * half + 2, :], ps[:])

            # stage 2: out[p, q], p-tiles mm = 0..1, both in one PSUM bank
            out_t = out_pool.tile([P, 2, NQ], F32, tag="out")
            ps2 = ps2_pool.tile([P, 2, NQ], F32, tag="ps2")
            for mm in range(2):
                for k in range(KW):
                    lhsT = tmpT_t[:, k, mm * P:(mm + 1) * P]
                    if windowed:
                        lo, hi = _window(k)
                        rhs = mht_t[:, k, lo:hi]
                        dst = ps2[:, mm, lo:hi]
                    else:
                        rhs = mht_t[:, k, :]
                        dst = ps2[:, mm, :]
                    nc.tensor.matmul(
                        dst, mmcast(lhsT), mmcast(rhs),
                        start=(k == 0), stop=(k == KW - 1),
                    )
            nc.scalar.copy(out_t[:], ps2[:])
            nc.sync.dma_start(out_d[i].rearrange("(m p) q -> p m q", p=P), out_t[:])


def build_nc(mode=MODE):
    nc = bacc.Bacc("TRN2", target_bir_lowering=False, debug=False,
                   num_devices=N_CORES)
    x_d = nc.dram_tensor("x", [C, H, W], F32, kind="ExternalInput").ap()
    mht_d = nc.dram_tensor("mht", [W, NQ], F32, kind="ExternalInput").ap()
    out_d = nc.dram_tensor("out", [C, 256, NQ], F32, kind="ExternalOutput").ap()
    with tile.TileContext(nc) as tc:
        _emit(nc, tc, x_d, mht_d, out_d, mode)
    nc.compile()
    return nc


_NC_CACHE = {}


def get_nc(mode=MODE):
    if mode not in _NC_CACHE:
        _NC_CACHE[mode] = build_nc(mode)
    return _NC_CACHE[mode]


def kernel(input, matrix_h, matrix_h_t):
    input = np.asarray(input, dtype=np.float32)
    mht = np.ascontiguousarray(np.asarray(matrix_h_t, dtype=np.float32))
    nc = get_nc()
    in_maps = [
        {"x": np.ascontiguousarray(input[b]), "mht": mht} for b in range(N_CORES)
    ]
    res = run_bass_kernel_spmd(nc, in_maps, core_ids=list(range(N_CORES)))
    return np.stack([res.results[b]["out"] for b in range(N_CORES)], axis=0)


# revision 4
# speedup vs baseline: 61.6052x; 61.6052x over previous
"""Trainium2 Bass kernel for 2D DWT low-pass (db2): out = mh @ x @ mht per (b,c).

Shapes (hardcoded): input [8, 64, 512, 512] f32, matrix_h [256, 512],
matrix_h_t [512, 256], output [8, 64, 256, 256] f32.

Sharding: data-parallel over the batch dim — core b processes input[b]
(64 images of 512x512); the small filter matrix is replicated.

Math (per image, H == W so matrix_h_t == matrix_h.T):
  stage 1: tmpT[w, p] = sum_h x[h, w] * mht[h, p]        (= (mh @ x).T)
  stage 2: out[p, q]  = sum_w tmpT[w, p] * mht[w, q]     (= mh @ x @ mht)
Both stages are PE matmuls out = lhsT.T @ rhs with the per-image tensor as
the stationary lhsT and the replicated mht as the moving rhs, so no
transposes are ever materialized.

mht is banded: rows 128k..128k+127 are nonzero only in columns
[max(0,64k-1), 64k+64]. Window modes restrict rhs/psum to that band,
cutting PE work ~4x (exact fp32 arithmetic, just skipping zeros).
"""

import os
import sys

sys.path.insert(0, "/opt/trn_rl_repo")

import numpy as np

import concourse.bass as bass
import concourse.tile as tile
from concourse import bacc, mybir
from concourse.bass_utils import run_bass_kernel_spmd

N_CORES = 8
C = 64          # images per core (channel dim; batch is the shard dim)
H = W = 512
P = 128         # SBUF partitions
KH = H // P     # 4 contraction chunks for stage 1
KW = W // P     # 4 contraction chunks for stage 2
NQ = 256        # output columns per matmul (full width)

F32 = mybir.dt.float32
F32R = mybir.dt.float32r

# mode: 'f32'  dense fp32 (exact, 4 cyc/row)
#       'f32w' windowed fp32 (exact, skips the zero band, 4 cyc/row on ~66 cols)
#       'f32r' dense float32r (1 cyc/row at N=256; HW numerics to be validated)
MODE = os.environ.get("DWT_MODE", "f32w")


def _window(k: int) -> tuple[int, int]:
    """Nonzero column range [lo, hi) of mht rows [128k, 128k+128)."""
    lo = max(0, 64 * k - 1)
    hi = min(NQ, 64 * k + 65)
    return lo, hi


def _emit(nc, tc, x_d, mht_d, out_d, mode, rounds=1):
    mm_dt = F32R if mode == "f32r" else F32
    windowed = mode == "f32w"

    def mmcast(ap):
        return ap.bitcast(mm_dt) if mm_dt is not F32 else ap

    with (
        tc.tile_pool(name="consts", bufs=1) as cpool,
        tc.tile_pool(name="img", bufs=3) as img_pool,
        tc.tile_pool(name="tmpT", bufs=2) as tmp_pool,
        tc.tile_pool(name="outp", bufs=3) as out_pool,
        tc.tile_pool(name="ps1", bufs=4, space=bass.MemorySpace.PSUM) as ps1_pool,
        tc.tile_pool(name="ps2", bufs=2, space=bass.MemorySpace.PSUM) as ps2_pool,
    ):
        # mht packed [128, KH, 256]: chunk k = mht[128k:128k+128, :]
        mht_t = cpool.tile([P, KH, NQ], F32)
        nc.sync.dma_start(mht_t[:], mht_d.rearrange("(k p) q -> p k q", p=P))

        def body():
            _emit_images(nc, tc, x_d, out_d, mht_t,
                         img_pool, tmp_pool, out_pool, ps1_pool, ps2_pool,
                         mmcast, windowed)

        if rounds == 1:
            body()
        else:
            # benchmark-only: repeat the whole workload `rounds` times in a
            # hardware loop; wall(R) - wall(1) isolates the HW exec time from
            # the fixed operand-relay/dispatch cost.
            with tc.For_i(0, rounds, 1):
                body()


def _emit_images(nc, tc, x_d, out_d, mht_t, img_pool, tmp_pool, out_pool,
                 ps1_pool, ps2_pool, mmcast, windowed):
        for i in range(C):
            img_t = img_pool.tile([P, KH, W], F32, tag="img")
            nc.sync.dma_start(img_t[:], x_d[i].rearrange("(k p) w -> p k w", p=P))

            # stage 1: tmpT[w, p], w-tiles m = 0..3, two per PSUM bank
            tmpT_t = tmp_pool.tile([P, KW, NQ], F32, tag="tmpT")
            for half in range(2):
                ps = ps1_pool.tile([P, 2, NQ], F32, tag="ps1")
                for ml in range(2):
                    m = 2 * half + ml
                    for k in range(KH):
                        lhsT = img_t[:, k, m * P:(m + 1) * P]
                        if windowed:
                            lo, hi = _window(k)
                            rhs = mht_t[:, k, lo:hi]
                            dst = ps[:, ml, lo:hi]
                        else:
                            rhs = mht_t[:, k, :]
                            dst = ps[:, ml, :]
                        nc.tensor.matmul(
                            dst, mmcast(lhsT), mmcast(rhs),
                            start=(k == 0), stop=(k == KH - 1),
                        )
                nc.vector.tensor_copy(tmpT_t[:, 2 * half:2 * half + 2, :], ps[:])

            # stage 2: out[p, q], p-tiles mm = 0..1, both in one PSUM bank
            out_t = out_pool.tile([P, 2, NQ], F32, tag="out")
            ps2 = ps2_pool.tile([P, 2, NQ], F32, tag="ps2")
            for mm in range(2):
                for k in range(KW):
                    lhsT = tmpT_t[:, k, mm * P:(mm + 1) * P]
                    if windowed:
                        lo, hi = _window(k)
                        rhs = mht_t[:, k, lo:hi]
                        dst = ps2[:, mm, lo:hi]
                    else:
                        rhs = mht_t[:, k, :]
                        dst = ps2[:, mm, :]
                    nc.tensor.matmul(
                        dst, mmcast(lhsT), mmcast(rhs),
                        start=(k == 0), stop=(k == KW - 1),
                    )
            nc.scalar.copy(out_t[:], ps2[:])
            nc.sync.dma_start(out_d[i].rearrange("(m p) q -> p m q", p=P), out_t[:])


def build_nc(mode=MODE, rounds=1):
    nc = bacc.Bacc("TRN2", target_bir_lowering=False, debug=False,
                   num_devices=N_CORES)
    x_d = nc.dram_tensor("x", [C, H, W], F32, kind="ExternalInput").ap()
    mht_d = nc.dram_tensor("mht", [W, NQ], F32, kind="ExternalInput").ap()
    out_d = nc.dram_tensor("out", [C, 256, NQ], F32, kind="ExternalOutput").ap()
    with tile.TileContext(nc) as tc:
        _emit(nc, tc, x_d, mht_d, out_d, mode, rounds=rounds)
    nc.compile()
    return nc


_NC_CACHE = {}


def get_nc(mode=MODE):
    if mode not in _NC_CACHE:
        _NC_CACHE[mode] = build_nc(mode)
    return _NC_CACHE[mode]


def kernel(input, matrix_h, matrix_h_t):
    input = np.asarray(input, dtype=np.float32)
    mht = np.ascontiguousarray(np.asarray(matrix_h_t, dtype=np.float32))
    nc = get_nc()
    in_maps = [
        {"x": np.ascontiguousarray(input[b]), "mht": mht} for b in range(N_CORES)
    ]
    res = run_bass_kernel_spmd(nc, in_maps, core_ids=list(range(N_CORES)))
    return np.stack([res.results[b]["out"] for b in range(N_CORES)], axis=0)


# revision 22
# speedup vs baseline: 209.8795x; 3.4068x over previous
"""Trainium2 Bass kernel for 2D DWT low-pass (db2): out = mh @ x @ mht per (b,c).

Shapes (hardcoded): input [8, 64, 512, 512] f32, matrix_h [256, 512],
matrix_h_t [512, 256], output [8, 64, 256, 256] f32.

Sharding: data-parallel over the batch dim — core b processes input[b]
(64 images of 512x512); the small filter matrix is replicated.

The filter matrices are banded 4-tap / stride-2 (mh[r, c] != 0 only for
c in [2r-1, 2r+2]; for H == W, matrix_h_t == matrix_h.T).

Best mode ('stencil3'), exact-fp32-quality arithmetic (measured rel err
~2.6e-7 vs the fp32 reference):
  stage 1 (H-direction): tmp[p, w] = sum_h mh[p, h] x[h, w] as PE matmuls
    with mht as the *stationary* operand. For output p-tile m (128 rows),
    the band only touches x row chunks 2m and 2m+1 (+1 corner row handled
    by a DVE fixup through a zero-padded scratch tile) -> 4 matmuls per
    image, N=512. The h2 tap is folded into the PSUM->SBUF copy on the
    Scalar engine (activation Copy with scale).
  stage 2 (W-direction): out[q] = sum_t h_t tmp[2q-1+t] as a 3-op DVE
    stencil along the free dim with fused multiply-add
    (scalar_tensor_tensor) on the h2-scaled tmp. No transposes anywhere.

The stencil taps are extracted from the matrix_h_t actually passed in
(so a degenerate, e.g. all-zero, matrix still produces correct output).
"""

import os
import sys

sys.path.insert(0, "/opt/trn_rl_repo")

import numpy as np

import concourse.bass as bass
import concourse.tile as tile
from concourse import bacc, mybir
from concourse.bass_utils import run_bass_kernel_spmd

N_CORES = 8
C = 64          # images per core (channel dim; batch is the shard dim)
H = W = 512
P = 128         # SBUF partitions
KH = H // P     # 4 row chunks
NQ = 256        # output columns

F32 = mybir.dt.float32
F32R = mybir.dt.float32r
MULT = mybir.AluOpType.mult
ADD = mybir.AluOpType.add

MODE = os.environ.get("DWT_MODE", "stencil3")


def _window(k: int) -> tuple[int, int]:
    """Nonzero column range [lo, hi) of mht rows [128k, 128k+128)."""
    lo = max(0, 64 * k - 1)
    hi = min(NQ, 64 * k + 65)
    return lo, hi


def _even(ap):
    """[p, n] view of elements 0,2,4,... of a [p, 2n] AP."""
    n2 = ap.shape[-1]
    return ap.rearrange("p (w t) -> p w t", t=2)[:, :, 0]


def _odd(ap):
    n2 = ap.shape[-1]
    return ap.rearrange("p (w t) -> p w t", t=2)[:, :, 1]


def _emit_stencil3(nc, tc, x_d, mht_d, out_d, taps, rounds):
    """stencil with: h2 folded into the PSUM->SBUF copy (ACT activation
    scale), 3-op stage-2 stencil on DVE using tap ratios, corner fixups on
    the otherwise-idle GpSimd engine."""
    h0, h1, h2, h3 = (float(t) for t in taps)
    # stage-2 reads the h2-scaled tmp; ratios recover the other taps
    r0 = h0 / h2 if h2 else 0.0
    r3 = h3 / h2 if h2 else 0.0
    with (
        tc.tile_pool(name="consts", bufs=1) as cpool,
        tc.tile_pool(name="img", bufs=3) as img_pool,
        tc.tile_pool(name="tmp", bufs=3) as tmp_pool,
        tc.tile_pool(name="outp", bufs=3) as out_pool,
        tc.tile_pool(name="ps1", bufs=4, space=bass.MemorySpace.PSUM) as ps1_pool,
    ):
        mht_t = cpool.tile([P, KH, NQ], F32)
        nc.sync.dma_start(mht_t[:], mht_d.rearrange("(k p) q -> p k q", p=P))

        zb = [cpool.tile([P, W], F32, tag=f"zb{z}", name=f"zb{z}")
              for z in range(2)]
        for z in range(2):
            nc.gpsimd.memset(zb[z][:], 0.0)

        def one_image(i):
            img_t = img_pool.tile([P, KH, W], F32, tag="img")
            nc.sync.dma_start(img_t[:], x_d[i].rearrange("(k p) w -> p k w", p=P))
            zb_i = zb[i % 2]
            nc.sync.dma_start(zb_i[127:128, :], x_d[i, 256:257, :])
            xr = img_pool.tile([1, W], F32, tag="xr")
            nc.sync.dma_start(xr[:], x_d[i, 255:256, :])

            out_t = out_pool.tile([P, 2, NQ], F32, tag="out")
            for m in range(2):
                ps = ps1_pool.tile([P, W], F32, tag="ps1")
                for j in range(2):
                    k = 2 * m + j
                    lhsT = mht_t[:, k, m * P:(m + 1) * P]
                    nc.tensor.matmul(ps[:], lhsT, img_t[:, k, :],
                                     start=(j == 0), stop=(j == 1))
                # tmp_s = h2 * tmp  (scale folded into the ACT copy)
                tmp_m = tmp_pool.tile([P, W], F32, tag="tmp")
                nc.scalar.activation(
                    tmp_m[:], ps[:],
                    mybir.ActivationFunctionType.Copy, scale=h2)
                # corner fixups on GpSimd (values pre-scaled by h2):
                if m == 0:
                    # tmp_s[127] += h2*h3 * x[256]
                    nc.vector.scalar_tensor_tensor(
                        tmp_m[:], zb_i[:], h2 * h3, tmp_m[:], MULT, ADD)
                else:
                    # tmp_s[128] += h2*h0 * x[255]
                    nc.vector.scalar_tensor_tensor(
                        tmp_m[0:1, :], xr[:], h2 * h0, tmp_m[0:1, :], MULT, ADD)

                # stage 2 on scaled tmp: out[q] = (h1/h2)*ev_s[q] + od_s[q]
                #   + (h0/h2)*od_s[q-1] + (h3/h2)*ev_s[q+1], all over tmp_s
                acc = out_t[:, m, :]
                nc.vector.scalar_tensor_tensor(
                    acc, _even(tmp_m[:]), h1 / h2 if h2 else 0.0,
                    _odd(tmp_m[:]), MULT, ADD)
                nc.vector.scalar_tensor_tensor(
                    acc[:, 1:], _even(tmp_m[:, 1:511]), r0,
                    acc[:, 1:], MULT, ADD)
                nc.vector.scalar_tensor_tensor(
                    acc[:, 0:255], _even(tmp_m[:, 2:512]), r3,
                    acc[:, 0:255], MULT, ADD)
            nc.sync.dma_start(out_d[i].rearrange("(m p) q -> p m q", p=P), out_t[:])

        def body():
            for i in range(C):
                one_image(i)

        if rounds == 1:
            body()
        else:
            with tc.For_i(0, rounds, 1):
                body()


def _emit_stencil4(nc, tc, x_d, mht_d, out_d, taps, rounds):
    """stencil3 + input/output DMA batched over pairs of images (2 MB in /
    512 KB out per dma_start) for higher HBM DMA efficiency."""
    h0, h1, h2, h3 = (float(t) for t in taps)
    r0 = h0 / h2 if h2 else 0.0
    r1 = h1 / h2 if h2 else 0.0
    r3 = h3 / h2 if h2 else 0.0
    with (
        tc.tile_pool(name="consts", bufs=1) as cpool,
        tc.tile_pool(name="img", bufs=3) as img_pool,
        tc.tile_pool(name="tmp", bufs=3) as tmp_pool,
        tc.tile_pool(name="outp", bufs=3) as out_pool,
        tc.tile_pool(name="ps1", bufs=4, space=bass.MemorySpace.PSUM) as ps1_pool,
    ):
        mht_t = cpool.tile([P, KH, NQ], F32)
        nc.sync.dma_start(mht_t[:], mht_d.rearrange("(k p) q -> p k q", p=P))

        zb = [cpool.tile([P, W], F32, tag=f"zb{z}", name=f"zb{z}")
              for z in range(2)]
        for z in range(2):
            nc.gpsimd.memset(zb[z][:], 0.0)

        def image_pair(i0):
            img_t = img_pool.tile([P, 2, KH, W], F32, tag="img")
            nc.sync.dma_start(
                img_t[:],
                x_d[i0:i0 + 2].rearrange("c (k p) w -> p c k w", p=P))
            # corner rows for both images: x[255] and x[256]
            xr = img_pool.tile([1, 2, 2, W], F32, tag="xr")
            nc.sync.dma_start(
                xr[:], x_d[i0:i0 + 2, 255:257, :].unsqueeze(0))
            out_t = out_pool.tile([P, 2, 2, NQ], F32, tag="out")
            for ci in range(2):
                i = i0 + ci
                zb_i = zb[ci]
                nc.sync.dma_start(zb_i[127:128, :], x_d[i, 256:257, :])
                for m in range(2):
                    ps = ps1_pool.tile([P, W], F32, tag="ps1")
                    for j in range(2):
                        k = 2 * m + j
                        lhsT = mht_t[:, k, m * P:(m + 1) * P]
                        nc.tensor.matmul(ps[:], lhsT, img_t[:, ci, k, :],
                                         start=(j == 0), stop=(j == 1))
                    tmp_m = tmp_pool.tile([P, W], F32, tag="tmp")
                    nc.scalar.activation(
                        tmp_m[:], ps[:],
                        mybir.ActivationFunctionType.Copy, scale=h2)
                    if m == 0:
                        # tmp_s[127] += h2*h3 * x[256]
                        nc.vector.scalar_tensor_tensor(
                            tmp_m[:], zb_i[:], h2 * h3, tmp_m[:], MULT, ADD)
                    else:
                        # tmp_s[128] += h2*h0 * x[255]
                        nc.vector.scalar_tensor_tensor(
                            tmp_m[0:1, :], xr[:, ci, 0, :], h2 * h0,
                            tmp_m[0:1, :], MULT, ADD)

                    acc = out_t[:, ci, m, :]
                    nc.vector.scalar_tensor_tensor(
                        acc, _even(tmp_m[:]), r1, _odd(tmp_m[:]), MULT, ADD)
                    nc.vector.scalar_tensor_tensor(
                        acc[:, 1:], _even(tmp_m[:, 1:511]), r0,
                        acc[:, 1:], MULT, ADD)
                    nc.vector.scalar_tensor_tensor(
                        acc[:, 0:255], _even(tmp_m[:, 2:512]), r3,
                        acc[:, 0:255], MULT, ADD)
            nc.sync.dma_start(
                out_d[i0:i0 + 2].rearrange("c (m p) q -> p c m q", p=P),
                out_t[:])

        def body():
            for i0 in range(0, C, 2):
                image_pair(i0)

        if rounds == 1:
            body()
        else:
            with tc.For_i(0, rounds, 1):
                body()


def _emit_stencil2(nc, tc, x_d, mht_d, out_d, taps, rounds):
    """Like stencil, but the matmul rhs stream is column-permuted (all even
    w's, then all odd w's) so tmp lands deinterleaved in PSUM and every
    stage-2 stencil operand is unit-stride (DVE 2x fp32 mode)."""
    h0, h1, h2, h3 = taps
    s2ops = int(os.environ.get("DWT_S2OPS", "4"))
    nofix = os.environ.get("DWT_NOFIX", "0") == "1"
    with (
        tc.tile_pool(name="consts", bufs=1) as cpool,
        tc.tile_pool(name="img", bufs=3) as img_pool,
        tc.tile_pool(name="tmp", bufs=3) as tmp_pool,
        tc.tile_pool(name="outp", bufs=3) as out_pool,
        tc.tile_pool(name="ps1", bufs=4, space=bass.MemorySpace.PSUM) as ps1_pool,
    ):
        mht_t = cpool.tile([P, KH, NQ], F32)
        nc.sync.dma_start(mht_t[:], mht_d.rearrange("(k p) q -> p k q", p=P))

        zb = [cpool.tile([P, 2, NQ], F32, tag=f"zb{z}", name=f"zb{z}")
              for z in range(2)]
        for z in range(2):
            nc.gpsimd.memset(zb[z][:], 0.0)

        def one_image(i):
            img_t = img_pool.tile([P, KH, W], F32, tag="img")
            nc.sync.dma_start(img_t[:], x_d[i].rearrange("(k p) w -> p k w", p=P))
            zb_i = zb[i % 2]
            if not nofix:
                # deinterleaved x[256] row into partition 127
                nc.sync.dma_start(
                    zb_i[127:128, :, :],
                    x_d[i, 256:257, :].rearrange("r (w t) -> r t w", t=2))
                xr = img_pool.tile([1, 2, NQ], F32, tag="xr")
                nc.sync.dma_start(
                    xr[:], x_d[i, 255:256, :].rearrange("r (w t) -> r t w", t=2))

            out_t = out_pool.tile([P, 2, NQ], F32, tag="out")
            for m in range(2):
                ps = ps1_pool.tile([P, 2, NQ], F32, tag="ps1")
                for j in range(2):
                    k = 2 * m + j
                    lhsT = mht_t[:, k, m * P:(m + 1) * P]
                    rhs = img_t[:, k, :].rearrange("p (w t) -> p t w", t=2)
                    nc.tensor.matmul(ps[:], lhsT, rhs,
                                     start=(j == 0), stop=(j == 1))
                tmp_m = tmp_pool.tile([P, 2, NQ], F32, tag="tmp")
                nc.scalar.copy(tmp_m[:], ps[:])
                if not nofix:
                    if m == 0:
                        # tmp[127] += h3 * x[256]
                        nc.vector.scalar_tensor_tensor(
                            tmp_m[:], zb_i[:], float(h3), tmp_m[:], MULT, ADD)
                    else:
                        # tmp[128] += h0 * x[255]
                        nc.vector.scalar_tensor_tensor(
                            tmp_m[0:1, :, :], xr[:], float(h0),
                            tmp_m[0:1, :, :], MULT, ADD)

                # stage 2, all unit-stride: ev = tmp_m[:,0,:], od = tmp_m[:,1,:]
                acc = out_t[:, m, :]
                nc.vector.tensor_scalar_mul(acc, tmp_m[:, 0, :], float(h1))
                if s2ops > 1:
                    nc.vector.scalar_tensor_tensor(
                        acc, tmp_m[:, 1, :], float(h2), acc, MULT, ADD)
                    nc.vector.scalar_tensor_tensor(
                        acc[:, 1:], tmp_m[:, 1, 0:255], float(h0),
                        acc[:, 1:], MULT, ADD)
                    nc.vector.scalar_tensor_tensor(
                        acc[:, 0:255], tmp_m[:, 0, 1:256], float(h3),
                        acc[:, 0:255], MULT, ADD)
            nc.sync.dma_start(out_d[i].rearrange("(m p) q -> p m q", p=P), out_t[:])

        def body():
            for i in range(C):
                one_image(i)

        if rounds == 1:
            body()
        else:
            with tc.For_i(0, rounds, 1):
                body()


def _emit_stencil(nc, tc, x_d, mht_d, out_d, taps, rounds):
    """stencil mode: 4 banded matmuls + DVE stencil per image."""
    h0, h1, h2, h3 = taps
    s2ops = int(os.environ.get("DWT_S2OPS", "4"))
    nofix = os.environ.get("DWT_NOFIX", "0") == "1"
    copy_eng = os.environ.get("DWT_COPY", "scalar")
    with (
        tc.tile_pool(name="consts", bufs=1) as cpool,
        tc.tile_pool(name="img", bufs=3) as img_pool,
        tc.tile_pool(name="tmp", bufs=3) as tmp_pool,
        tc.tile_pool(name="outp", bufs=3) as out_pool,
        tc.tile_pool(name="ps1", bufs=4, space=bass.MemorySpace.PSUM) as ps1_pool,
    ):
        # weights: mht chunk k rows, p-columns of tile m -> [128, 128] blocks
        # W[m][j] = mht[128*(2m+j) : ..+128, 128m : 128m+128], j in {0, 1}
        mht_t = cpool.tile([P, KH, NQ], F32)
        nc.sync.dma_start(mht_t[:], mht_d.rearrange("(k p) q -> p k q", p=P))

        # Engine APs need 32-aligned partition bases and TensorScalarPtr needs
        # equal base partitions on its SBUF operands, so the p=127 corner
        # (+= h3 * x[256]) goes through zero-padded [P, W] scratch tiles:
        # x[256] lands in partition 127 via DMA (no alignment limits there);
        # partitions 0..126 stay zero so the fused multiply-add over the whole
        # tmp tile only changes partition 127. Two tiles, used alternately, so
        # consecutive images don't serialize on one buffer.
        zb = [cpool.tile([P, W], F32, tag=f"zb{z}", name=f"zb{z}")
              for z in range(2)]
        for z in range(2):
            nc.gpsimd.memset(zb[z][:], 0.0)

        def one_image(i):
            img_t = img_pool.tile([P, KH, W], F32, tag="img")
            nc.sync.dma_start(img_t[:], x_d[i].rearrange("(k p) w -> p k w", p=P))
            zb_i = zb[i % 2]
            if not nofix:
                nc.sync.dma_start(zb_i[127:128, :], x_d[i, 256:257, :])
                # x[255] for the p=128 corner (partition 0 of tile 1: aligned)
                xr = img_pool.tile([1, W], F32, tag="xr")
                nc.sync.dma_start(xr[:], x_d[i, 255:256, :])

            out_t = out_pool.tile([P, 2, NQ], F32, tag="out")
            for m in range(2):
                # ---- stage 1: tmp[128m:128m+128, :] ----
                ps = ps1_pool.tile([P, W], F32, tag="ps1")
                for j in range(2):
                    k = 2 * m + j
                    lhsT = mht_t[:, k, m * P:(m + 1) * P]
                    nc.tensor.matmul(ps[:], lhsT, img_t[:, k, :],
                                     start=(j == 0), stop=(j == 1))
                tmp_m = tmp_pool.tile([P, W], F32, tag="tmp")
                if copy_eng == "vector":
                    nc.vector.tensor_copy(tmp_m[:], ps[:])
                else:
                    nc.scalar.copy(tmp_m[:], ps[:])
                # band corner fixups:
                if not nofix:
                    if m == 0:
                        # tmp[127] += h3 * x[256]
                        nc.vector.scalar_tensor_tensor(
                            tmp_m[:], zb_i[:], float(h3),
                            tmp_m[:], MULT, ADD)
                    else:
                        # tmp[128] += h0 * x[255]
                        nc.vector.scalar_tensor_tensor(
                            tmp_m[0:1, :], xr[:], float(h0),
                            tmp_m[0:1, :], MULT, ADD)

                # ---- stage 2: out rows 128m..128m+127 ----
                acc = out_t[:, m, :]
                # q=0..255: out[q] = h0*tmp[2q-1] + h1*tmp[2q] + h2*tmp[2q+1]
                #                  + h3*tmp[2q+2]   (h0@q=0, h3@q=255 clipped)
                nc.vector.tensor_scalar_mul(acc, _even(tmp_m[:]), float(h1))
                if s2ops > 1:
                    nc.vector.scalar_tensor_tensor(
                        acc, _odd(tmp_m[:]), float(h2), acc, MULT, ADD)
                    nc.vector.scalar_tensor_tensor(
                        acc[:, 1:], _even(tmp_m[:, 1:511]), float(h0),
                        acc[:, 1:], MULT, ADD)
                    nc.vector.scalar_tensor_tensor(
                        acc[:, 0:255], _even(tmp_m[:, 2:512]), float(h3),
                        acc[:, 0:255], MULT, ADD)
            nc.sync.dma_start(out_d[i].rearrange("(m p) q -> p m q", p=P), out_t[:])

        def body():
            for i in range(C):
                one_image(i)

        if rounds == 1:
            body()
        else:
            with tc.For_i(0, rounds, 1):
                body()


def _emit_matmul(nc, tc, x_d, mht_d, out_d, mode, rounds):
    """all-matmul scheme X: per-image tensor stationary, mht moving."""
    mm_dt = F32R if mode == "f32r" else F32
    windowed = mode == "f32w"

    def mmcast(ap):
        return ap.bitcast(mm_dt) if mm_dt is not F32 else ap

    with (
        tc.tile_pool(name="consts", bufs=1) as cpool,
        tc.tile_pool(name="img", bufs=3) as img_pool,
        tc.tile_pool(name="tmpT", bufs=2) as tmp_pool,
        tc.tile_pool(name="outp", bufs=3) as out_pool,
        tc.tile_pool(name="ps1", bufs=4, space=bass.MemorySpace.PSUM) as ps1_pool,
        tc.tile_pool(name="ps2", bufs=2, space=bass.MemorySpace.PSUM) as ps2_pool,
    ):
        mht_t = cpool.tile([P, KH, NQ], F32)
        nc.sync.dma_start(mht_t[:], mht_d.rearrange("(k p) q -> p k q", p=P))

        def one_image(i):
            img_t = img_pool.tile([P, KH, W], F32, tag="img")
            nc.sync.dma_start(img_t[:], x_d[i].rearrange("(k p) w -> p k w", p=P))

            # stage 1: tmpT[w, p], w-tiles m = 0..3, two per PSUM bank
            tmpT_t = tmp_pool.tile([P, KH, NQ], F32, tag="tmpT")
            for half in range(2):
                ps = ps1_pool.tile([P, 2, NQ], F32, tag="ps1")
                for ml in range(2):
                    m = 2 * half + ml
                    for k in range(KH):
                        lhsT = img_t[:, k, m * P:(m + 1) * P]
                        if windowed:
                            lo, hi = _window(k)
                            rhs = mht_t[:, k, lo:hi]
                            dst = ps[:, ml, lo:hi]
                        else:
                            rhs = mht_t[:, k, :]
                            dst = ps[:, ml, :]
                        nc.tensor.matmul(
                            dst, mmcast(lhsT), mmcast(rhs),
                            start=(k == 0), stop=(k == KH - 1),
                        )
                nc.vector.tensor_copy(tmpT_t[:, 2 * half:2 * half + 2, :], ps[:])

            # stage 2: out[p, q], p-tiles mm = 0..1, both in one PSUM bank
            out_t = out_pool.tile([P, 2, NQ], F32, tag="out")
            ps2 = ps2_pool.tile([P, 2, NQ], F32, tag="ps2")
            for mm in range(2):
                for k in range(KH):
                    lhsT = tmpT_t[:, k, mm * P:(mm + 1) * P]
                    if windowed:
                        lo, hi = _window(k)
                        rhs = mht_t[:, k, lo:hi]
                        dst = ps2[:, mm, lo:hi]
                    else:
                        rhs = mht_t[:, k, :]
                        dst = ps2[:, mm, :]
                    nc.tensor.matmul(
                        dst, mmcast(lhsT), mmcast(rhs),
                        start=(k == 0), stop=(k == KH - 1),
                    )
            nc.scalar.copy(out_t[:], ps2[:])
            nc.sync.dma_start(out_d[i].rearrange("(m p) q -> p m q", p=P), out_t[:])

        def body():
            for i in range(C):
                one_image(i)

        if rounds == 1:
            body()
        else:
            with tc.For_i(0, rounds, 1):
                body()


def build_nc(mode=MODE, rounds=1, taps=(0.0, 0.0, 0.0, 0.0)):
    nc = bacc.Bacc("TRN2", target_bir_lowering=False, debug=False,
                   num_devices=N_CORES)
    x_d = nc.dram_tensor("x", [C, H, W], F32, kind="ExternalInput").ap()
    mht_d = nc.dram_tensor("mht", [W, NQ], F32, kind="ExternalInput").ap()
    out_d = nc.dram_tensor("out", [C, 256, NQ], F32, kind="ExternalOutput").ap()
    with tile.TileContext(nc) as tc:
        if mode == "stencil":
            _emit_stencil(nc, tc, x_d, mht_d, out_d, taps, rounds)
        elif mode == "stencil2":
            _emit_stencil2(nc, tc, x_d, mht_d, out_d, taps, rounds)
        elif mode == "stencil3":
            _emit_stencil3(nc, tc, x_d, mht_d, out_d, taps, rounds)
        elif mode == "stencil4":
            _emit_stencil4(nc, tc, x_d, mht_d, out_d, taps, rounds)
        else:
            _emit_matmul(nc, tc, x_d, mht_d, out_d, mode, rounds)
    nc.compile()
    return nc


def extract_taps(mht: np.ndarray) -> tuple:
    """Band taps from the actual filter matrix: mht[2r-1+t, r] = h_t.
    Row r=1 keeps all four taps in-range."""
    return tuple(float(mht[1 + t, 1]) for t in range(4))


_NC_CACHE = {}


def get_nc(mode=MODE, rounds=1, taps=(0.0, 0.0, 0.0, 0.0)):
    key = (mode, rounds, taps if mode == "stencil" else None)
    if key not in _NC_CACHE:
        _NC_CACHE[key] = build_nc(mode, rounds=rounds, taps=taps)
    return _NC_CACHE[key]


def kernel(input, matrix_h, matrix_h_t):
    input = np.asarray(input, dtype=np.float32)
    mht = np.ascontiguousarray(np.asarray(matrix_h_t, dtype=np.float32))
    nc = get_nc(MODE, 1, extract_taps(mht))
    in_maps = [
        {"x": np.ascontiguousarray(input[b]), "mht": mht} for b in range(N_CORES)
    ]
    res = run_bass_kernel_spmd(nc, in_maps, core_ids=list(range(N_CORES)))
    return np.stack([res.results[b]["out"] for b in range(N_CORES)], axis=0)


# revision 30
# speedup vs baseline: 230.0354x; 1.0960x over previous
"""Trainium2 Bass kernel for 2D DWT low-pass (db2): out = mh @ x @ mht per (b,c).

Shapes (hardcoded): input [8, 64, 512, 512] f32, matrix_h [256, 512],
matrix_h_t [512, 256], output [8, 64, 256, 256] f32.

Sharding: data-parallel over the batch dim — core b processes input[b]
(64 images of 512x512); the small filter matrix is replicated.

The filter matrices are banded 4-tap / stride-2 (mh[r, c] != 0 only for
c in [2r-1, 2r+2]; for H == W, matrix_h_t == matrix_h.T).

Best mode ('stencil6'), exact-fp32-quality arithmetic (measured rel err
~2.6e-7 vs the fp32 reference):
  stage 1 (H-direction): tmp[p, w] = sum_h mh[p, h] x[h, w] as PE matmuls
    with mht as the *stationary* operand. For output p-tile m (128 rows),
    the band only touches x row chunks 2m and 2m+1 (+1 corner row handled
    by a DVE fixup through a zero-padded scratch tile) -> 4 matmuls per
    image, N=512. The h2 tap is folded into the PSUM->SBUF copy on the
    Scalar engine (activation Copy with scale).
  stage 2 (W-direction): out[q] = sum_t h_t tmp[2q-1+t] as a 3-op DVE
    stencil along the free dim with fused multiply-add
    (scalar_tensor_tensor) on the h2-scaled tmp. No transposes anywhere.
  The band-corner fixups run as 2 KB SBUF->SBUF accumulate-DMAs on the
    software DGE (GpSimd), fed by once-per-pass batch-pre-scaled corner
    rows, keeping them off the DVE critical path.

The stencil taps are extracted from the matrix_h_t actually passed in
(so a degenerate, e.g. all-zero, matrix still produces correct output).
"""

import os
import sys

sys.path.insert(0, "/opt/trn_rl_repo")

import numpy as np

import concourse.bass as bass
import concourse.tile as tile
from concourse import bacc, mybir
from concourse.bass_utils import run_bass_kernel_spmd

N_CORES = 8
C = 64          # images per core (channel dim; batch is the shard dim)
H = W = 512
P = 128         # SBUF partitions
KH = H // P     # 4 row chunks
NQ = 256        # output columns

F32 = mybir.dt.float32
F32R = mybir.dt.float32r
MULT = mybir.AluOpType.mult
ADD = mybir.AluOpType.add

MODE = os.environ.get("DWT_MODE", "stencil6")


def _window(k: int) -> tuple[int, int]:
    """Nonzero column range [lo, hi) of mht rows [128k, 128k+128)."""
    lo = max(0, 64 * k - 1)
    hi = min(NQ, 64 * k + 65)
    return lo, hi


def _even(ap):
    """[p, n] view of elements 0,2,4,... of a [p, 2n] AP."""
    n2 = ap.shape[-1]
    return ap.rearrange("p (w t) -> p w t", t=2)[:, :, 0]


def _odd(ap):
    n2 = ap.shape[-1]
    return ap.rearrange("p (w t) -> p w t", t=2)[:, :, 1]


def _emit_stencil3(nc, tc, x_d, mht_d, out_d, taps, rounds):
    """stencil with: h2 folded into the PSUM->SBUF copy (ACT activation
    scale), 3-op stage-2 stencil on DVE using tap ratios, corner fixups on
    the otherwise-idle GpSimd engine."""
    h0, h1, h2, h3 = (float(t) for t in taps)
    # stage-2 reads the h2-scaled tmp; ratios recover the other taps
    r0 = h0 / h2 if h2 else 0.0
    r3 = h3 / h2 if h2 else 0.0
    ib = int(os.environ.get("DWT_IB", "3"))
    pb = int(os.environ.get("DWT_PB", "4"))
    with (
        tc.tile_pool(name="consts", bufs=1) as cpool,
        tc.tile_pool(name="img", bufs=ib) as img_pool,
        tc.tile_pool(name="tmp", bufs=ib) as tmp_pool,
        tc.tile_pool(name="outp", bufs=ib) as out_pool,
        tc.tile_pool(name="ps1", bufs=pb, space=bass.MemorySpace.PSUM) as ps1_pool,
    ):
        mht_t = cpool.tile([P, KH, NQ], F32)
        nc.sync.dma_start(mht_t[:], mht_d.rearrange("(k p) q -> p k q", p=P))

        zb = [cpool.tile([P, W], F32, tag=f"zb{z}", name=f"zb{z}")
              for z in range(2)]
        for z in range(2):
            nc.gpsimd.memset(zb[z][:], 0.0)

        def one_image(i):
            img_t = img_pool.tile([P, KH, W], F32, tag="img")
            nc.sync.dma_start(img_t[:], x_d[i].rearrange("(k p) w -> p k w", p=P))
            zb_i = zb[i % 2]
            nc.sync.dma_start(zb_i[127:128, :], x_d[i, 256:257, :])
            xr = img_pool.tile([1, W], F32, tag="xr")
            nc.sync.dma_start(xr[:], x_d[i, 255:256, :])

            out_t = out_pool.tile([P, 2, NQ], F32, tag="out")
            for m in range(2):
                ps = ps1_pool.tile([P, W], F32, tag="ps1")
                for j in range(2):
                    k = 2 * m + j
                    lhsT = mht_t[:, k, m * P:(m + 1) * P]
                    nc.tensor.matmul(ps[:], lhsT, img_t[:, k, :],
                                     start=(j == 0), stop=(j == 1))
                # tmp_s = h2 * tmp  (scale folded into the ACT copy)
                tmp_m = tmp_pool.tile([P, W], F32, tag="tmp")
                nc.scalar.activation(
                    tmp_m[:], ps[:],
                    mybir.ActivationFunctionType.Copy, scale=h2)
                # corner fixups on GpSimd (values pre-scaled by h2):
                if m == 0:
                    # tmp_s[127] += h2*h3 * x[256]
                    nc.vector.scalar_tensor_tensor(
                        tmp_m[:], zb_i[:], h2 * h3, tmp_m[:], MULT, ADD)
                else:
                    # tmp_s[128] += h2*h0 * x[255]
                    nc.vector.scalar_tensor_tensor(
                        tmp_m[0:1, :], xr[:], h2 * h0, tmp_m[0:1, :], MULT, ADD)

                # stage 2 on scaled tmp: out[q] = (h1/h2)*ev_s[q] + od_s[q]
                #   + (h0/h2)*od_s[q-1] + (h3/h2)*ev_s[q+1], all over tmp_s
                acc = out_t[:, m, :]
                nc.vector.scalar_tensor_tensor(
                    acc, _even(tmp_m[:]), h1 / h2 if h2 else 0.0,
                    _odd(tmp_m[:]), MULT, ADD)
                nc.vector.scalar_tensor_tensor(
                    acc[:, 1:], _even(tmp_m[:, 1:511]), r0,
                    acc[:, 1:], MULT, ADD)
                nc.vector.scalar_tensor_tensor(
                    acc[:, 0:255], _even(tmp_m[:, 2:512]), r3,
                    acc[:, 0:255], MULT, ADD)
            nc.sync.dma_start(out_d[i].rearrange("(m p) q -> p m q", p=P), out_t[:])

        def body():
            for i in range(C):
                one_image(i)

        if rounds == 1:
            body()
        else:
            with tc.For_i(0, rounds, 1):
                body()


def _emit_stencil5(nc, tc, x_d, mht_d, out_d, taps, rounds):
    """stencil3, but the PSUM->SBUF copies deinterleave tmp (even/odd w
    halves via strided ACT reads, unit-stride writes) so every DVE stage-2
    and fixup operand is unit-stride (fp32 2x mode)."""
    h0, h1, h2, h3 = (float(t) for t in taps)
    r0 = h0 / h2 if h2 else 0.0
    r1 = h1 / h2 if h2 else 0.0
    r3 = h3 / h2 if h2 else 0.0
    ib = int(os.environ.get("DWT_IB", "3"))
    pb = int(os.environ.get("DWT_PB", "4"))
    with (
        tc.tile_pool(name="consts", bufs=1) as cpool,
        tc.tile_pool(name="img", bufs=ib) as img_pool,
        tc.tile_pool(name="tmp", bufs=ib) as tmp_pool,
        tc.tile_pool(name="outp", bufs=ib) as out_pool,
        tc.tile_pool(name="ps1", bufs=pb, space=bass.MemorySpace.PSUM) as ps1_pool,
    ):
        mht_t = cpool.tile([P, KH, NQ], F32)
        nc.sync.dma_start(mht_t[:], mht_d.rearrange("(k p) q -> p k q", p=P))

        zb = [cpool.tile([P, 2, NQ], F32, tag=f"zb{z}", name=f"zb{z}")
              for z in range(2)]
        for z in range(2):
            nc.gpsimd.memset(zb[z][:], 0.0)

        def one_image(i):
            img_t = img_pool.tile([P, KH, W], F32, tag="img")
            nc.sync.dma_start(img_t[:], x_d[i].rearrange("(k p) w -> p k w", p=P))
            zb_i = zb[i % 2]
            # deinterleaved corner rows (ev half then od half)
            nc.sync.dma_start(
                zb_i[127:128, :, :],
                x_d[i, 256:257, :].rearrange("r (w t) -> r t w", t=2))
            xr = img_pool.tile([1, 2, NQ], F32, tag="xr")
            nc.sync.dma_start(
                xr[:], x_d[i, 255:256, :].rearrange("r (w t) -> r t w", t=2))

            out_t = out_pool.tile([P, 2, NQ], F32, tag="out")
            for m in range(2):
                ps = ps1_pool.tile([P, W], F32, tag="ps1")
                for j in range(2):
                    k = 2 * m + j
                    lhsT = mht_t[:, k, m * P:(m + 1) * P]
                    nc.tensor.matmul(ps[:], lhsT, img_t[:, k, :],
                                     start=(j == 0), stop=(j == 1))
                # deinterleaving scaled copies: tmp_m[:,0,:] = h2*tmp[2q],
                # tmp_m[:,1,:] = h2*tmp[2q+1]
                tmp_m = tmp_pool.tile([P, 2, NQ], F32, tag="tmp")
                nc.scalar.activation(
                    tmp_m[:, 0, :], _even(ps[:]),
                    mybir.ActivationFunctionType.Copy, scale=h2)
                nc.scalar.activation(
                    tmp_m[:, 1, :], _odd(ps[:]),
                    mybir.ActivationFunctionType.Copy, scale=h2)
                if m == 0:
                    # tmp_s[127] += h2*h3 * x[256]
                    nc.vector.scalar_tensor_tensor(
                        tmp_m[:], zb_i[:], h2 * h3, tmp_m[:], MULT, ADD)
                else:
                    # tmp_s[128] += h2*h0 * x[255]
                    nc.vector.scalar_tensor_tensor(
                        tmp_m[0:1, :, :], xr[:], h2 * h0,
                        tmp_m[0:1, :, :], MULT, ADD)

                acc = out_t[:, m, :]
                nc.vector.scalar_tensor_tensor(
                    acc, tmp_m[:, 0, :], r1, tmp_m[:, 1, :], MULT, ADD)
                nc.vector.scalar_tensor_tensor(
                    acc[:, 1:], tmp_m[:, 1, 0:255], r0, acc[:, 1:], MULT, ADD)
                nc.vector.scalar_tensor_tensor(
                    acc[:, 0:255], tmp_m[:, 0, 1:256], r3,
                    acc[:, 0:255], MULT, ADD)
            nc.sync.dma_start(out_d[i].rearrange("(m p) q -> p m q", p=P), out_t[:])

        def body():
            for i in range(C):
                one_image(i)

        if rounds == 1:
            body()
        else:
            with tc.For_i(0, rounds, 1):
                body()


def _emit_stencil6(nc, tc, x_d, mht_d, out_d, taps, rounds):
    """stencil3 + corner fixups as GpSimd accumulate-DMAs: the corner rows
    x[:,255,:] / x[:,256,:] of all images are loaded deinterleaved once per
    round, pre-scaled in two batched DVE ops, and added into the tmp tiles
    with 2 KB SBUF->SBUF accum DMAs on the otherwise-idle software DGE."""
    h0, h1, h2, h3 = (float(t) for t in taps)
    r0 = h0 / h2 if h2 else 0.0
    r1 = h1 / h2 if h2 else 0.0
    r3 = h3 / h2 if h2 else 0.0
    ib = int(os.environ.get("DWT_IB", "3"))
    pb = int(os.environ.get("DWT_PB", "4"))
    with (
        tc.tile_pool(name="consts", bufs=1) as cpool,
        tc.tile_pool(name="img", bufs=ib) as img_pool,
        tc.tile_pool(name="tmp", bufs=ib) as tmp_pool,
        tc.tile_pool(name="outp", bufs=ib) as out_pool,
        tc.tile_pool(name="corner", bufs=2) as corner_pool,
        tc.tile_pool(name="ps1", bufs=pb, space=bass.MemorySpace.PSUM) as ps1_pool,
    ):
        mht_t = cpool.tile([P, KH, NQ], F32)
        nc.sync.dma_start(mht_t[:], mht_d.rearrange("(k p) q -> p k q", p=P))

        def body():
            # corner rows of all images: partition c = image c; [c, r, w]
            xc = corner_pool.tile([C, 2, W], F32, tag="xc")
            nc.sync.dma_start(xc[:], x_d[:, 255:257, :])
            sc = corner_pool.tile([C, 2, W], F32, tag="sc")
            # row 255 -> tmp[128] fixup (scale h2*h0); row 256 -> tmp[127]
            nc.vector.tensor_scalar_mul(sc[:, 0], xc[:, 0], h2 * h0)
            nc.vector.tensor_scalar_mul(sc[:, 1], xc[:, 1], h2 * h3)

            for i in range(C):
                img_t = img_pool.tile([P, KH, W], F32, tag="img")
                nc.sync.dma_start(img_t[:],
                                  x_d[i].rearrange("(k p) w -> p k w", p=P))

                out_t = out_pool.tile([P, 2, NQ], F32, tag="out")
                for m in range(2):
                    ps = ps1_pool.tile([P, W], F32, tag="ps1")
                    for j in range(2):
                        k = 2 * m + j
                        lhsT = mht_t[:, k, m * P:(m + 1) * P]
                        nc.tensor.matmul(ps[:], lhsT, img_t[:, k, :],
                                         start=(j == 0), stop=(j == 1))
                    tmp_m = tmp_pool.tile([P, W], F32, tag="tmp")
                    nc.scalar.activation(
                        tmp_m[:], ps[:],
                        mybir.ActivationFunctionType.Copy, scale=h2)
                    if m == 0:
                        # tmp_s[127] += h2*h3 * x[256]
                        nc.gpsimd.dma_start(tmp_m[127:128, :],
                                            sc[i:i + 1, 1, :],
                                            accum_op=ADD)
                    else:
                        # tmp_s[128] += h2*h0 * x[255]
                        nc.gpsimd.dma_start(tmp_m[0:1, :],
                                            sc[i:i + 1, 0, :],
                                            accum_op=ADD)

                    acc = out_t[:, m, :]
                    nc.vector.scalar_tensor_tensor(
                        acc, _even(tmp_m[:]), r1, _odd(tmp_m[:]), MULT, ADD)
                    nc.vector.scalar_tensor_tensor(
                        acc[:, 1:], _even(tmp_m[:, 1:511]), r0,
                        acc[:, 1:], MULT, ADD)
                    nc.vector.scalar_tensor_tensor(
                        acc[:, 0:255], _even(tmp_m[:, 2:512]), r3,
                        acc[:, 0:255], MULT, ADD)
                nc.sync.dma_start(out_d[i].rearrange("(m p) q -> p m q", p=P),
                                  out_t[:])

        if rounds == 1:
            body()
        else:
            with tc.For_i(0, rounds, 1):
                body()


def _emit_stencil7(nc, tc, x_d, mht_d, out_d, taps, rounds):
    """stencil6 + output DMA batched over groups of 4 images (1 MB per
    store) for better HBM store efficiency; input stays per-image so
    compute never waits on a multi-image load."""
    h0, h1, h2, h3 = (float(t) for t in taps)
    r0 = h0 / h2 if h2 else 0.0
    r1 = h1 / h2 if h2 else 0.0
    r3 = h3 / h2 if h2 else 0.0
    ib = int(os.environ.get("DWT_IB", "3"))
    pb = int(os.environ.get("DWT_PB", "4"))
    GB = 4  # images per output-DMA group
    with (
        tc.tile_pool(name="consts", bufs=1) as cpool,
        tc.tile_pool(name="img", bufs=ib) as img_pool,
        tc.tile_pool(name="tmp", bufs=ib) as tmp_pool,
        tc.tile_pool(name="outp", bufs=2) as out_pool,
        tc.tile_pool(name="corner", bufs=2) as corner_pool,
        tc.tile_pool(name="ps1", bufs=pb, space=bass.MemorySpace.PSUM) as ps1_pool,
    ):
        mht_t = cpool.tile([P, KH, NQ], F32)
        nc.sync.dma_start(mht_t[:], mht_d.rearrange("(k p) q -> p k q", p=P))

        def body():
            xc = corner_pool.tile([C, 2, W], F32, tag="xc")
            nc.sync.dma_start(xc[:], x_d[:, 255:257, :])
            sc = corner_pool.tile([C, 2, W], F32, tag="sc")
            nc.vector.tensor_scalar_mul(sc[:, 0], xc[:, 0], h2 * h0)
            nc.vector.tensor_scalar_mul(sc[:, 1], xc[:, 1], h2 * h3)

            for i0 in range(0, C, GB):
                out_t = out_pool.tile([P, GB, 2, NQ], F32, tag="out")
                for ci in range(GB):
                    i = i0 + ci
                    img_t = img_pool.tile([P, KH, W], F32, tag="img")
                    nc.sync.dma_start(
                        img_t[:], x_d[i].rearrange("(k p) w -> p k w", p=P))
                    for m in range(2):
                        ps = ps1_pool.tile([P, W], F32, tag="ps1")
                        for j in range(2):
                            k = 2 * m + j
                            lhsT = mht_t[:, k, m * P:(m + 1) * P]
                            nc.tensor.matmul(ps[:], lhsT, img_t[:, k, :],
                                             start=(j == 0), stop=(j == 1))
                        tmp_m = tmp_pool.tile([P, W], F32, tag="tmp")
                        nc.scalar.activation(
                            tmp_m[:], ps[:],
                            mybir.ActivationFunctionType.Copy, scale=h2)
                        if m == 0:
                            nc.gpsimd.dma_start(tmp_m[127:128, :],
                                                sc[i:i + 1, 1, :],
                                                accum_op=ADD)
                        else:
                            nc.gpsimd.dma_start(tmp_m[0:1, :],
                                                sc[i:i + 1, 0, :],
                                                accum_op=ADD)

                        acc = out_t[:, ci, m, :]
                        nc.vector.scalar_tensor_tensor(
                            acc, _even(tmp_m[:]), r1, _odd(tmp_m[:]),
                            MULT, ADD)
                        nc.vector.scalar_tensor_tensor(
                            acc[:, 1:], _even(tmp_m[:, 1:511]), r0,
                            acc[:, 1:], MULT, ADD)
                        nc.vector.scalar_tensor_tensor(
                            acc[:, 0:255], _even(tmp_m[:, 2:512]), r3,
                            acc[:, 0:255], MULT, ADD)
                nc.sync.dma_start(
                    out_d[i0:i0 + GB].rearrange("c (m p) q -> p (c m) q", p=P),
                    out_t[:].rearrange("p c m q -> p (c m) q"))

        if rounds == 1:
            body()
        else:
            with tc.For_i(0, rounds, 1):
                body()


def _emit_stencil4(nc, tc, x_d, mht_d, out_d, taps, rounds):
    """stencil3 + input/output DMA batched over pairs of images (2 MB in /
    512 KB out per dma_start) for higher HBM DMA efficiency."""
    h0, h1, h2, h3 = (float(t) for t in taps)
    r0 = h0 / h2 if h2 else 0.0
    r1 = h1 / h2 if h2 else 0.0
    r3 = h3 / h2 if h2 else 0.0
    with (
        tc.tile_pool(name="consts", bufs=1) as cpool,
        tc.tile_pool(name="img", bufs=3) as img_pool,
        tc.tile_pool(name="tmp", bufs=3) as tmp_pool,
        tc.tile_pool(name="outp", bufs=3) as out_pool,
        tc.tile_pool(name="ps1", bufs=4, space=bass.MemorySpace.PSUM) as ps1_pool,
    ):
        mht_t = cpool.tile([P, KH, NQ], F32)
        nc.sync.dma_start(mht_t[:], mht_d.rearrange("(k p) q -> p k q", p=P))

        zb = [cpool.tile([P, W], F32, tag=f"zb{z}", name=f"zb{z}")
              for z in range(2)]
        for z in range(2):
            nc.gpsimd.memset(zb[z][:], 0.0)

        def image_pair(i0):
            img_t = img_pool.tile([P, 2, KH, W], F32, tag="img")
            nc.sync.dma_start(
                img_t[:],
                x_d[i0:i0 + 2].rearrange("c (k p) w -> p c k w", p=P))
            # corner rows for both images: x[255] and x[256]
            xr = img_pool.tile([1, 2, 2, W], F32, tag="xr")
            nc.sync.dma_start(
                xr[:], x_d[i0:i0 + 2, 255:257, :].unsqueeze(0))
            out_t = out_pool.tile([P, 2, 2, NQ], F32, tag="out")
            for ci in range(2):
                i = i0 + ci
                zb_i = zb[ci]
                nc.sync.dma_start(zb_i[127:128, :], x_d[i, 256:257, :])
                for m in range(2):
                    ps = ps1_pool.tile([P, W], F32, tag="ps1")
                    for j in range(2):
                        k = 2 * m + j
                        lhsT = mht_t[:, k, m * P:(m + 1) * P]
                        nc.tensor.matmul(ps[:], lhsT, img_t[:, ci, k, :],
                                         start=(j == 0), stop=(j == 1))
                    tmp_m = tmp_pool.tile([P, W], F32, tag="tmp")
                    nc.scalar.activation(
                        tmp_m[:], ps[:],
                        mybir.ActivationFunctionType.Copy, scale=h2)
                    if m == 0:
                        # tmp_s[127] += h2*h3 * x[256]
                        nc.vector.scalar_tensor_tensor(
                            tmp_m[:], zb_i[:], h2 * h3, tmp_m[:], MULT, ADD)
                    else:
                        # tmp_s[128] += h2*h0 * x[255]
                        nc.vector.scalar_tensor_tensor(
                            tmp_m[0:1, :], xr[:, ci, 0, :], h2 * h0,
                            tmp_m[0:1, :], MULT, ADD)

                    acc = out_t[:, ci, m, :]
                    nc.vector.scalar_tensor_tensor(
                        acc, _even(tmp_m[:]), r1, _odd(tmp_m[:]), MULT, ADD)
                    nc.vector.scalar_tensor_tensor(
                        acc[:, 1:], _even(tmp_m[:, 1:511]), r0,
                        acc[:, 1:], MULT, ADD)
                    nc.vector.scalar_tensor_tensor(
                        acc[:, 0:255], _even(tmp_m[:, 2:512]), r3,
                        acc[:, 0:255], MULT, ADD)
            nc.sync.dma_start(
                out_d[i0:i0 + 2].rearrange("c (m p) q -> p c m q", p=P),
                out_t[:])

        def body():
            for i0 in range(0, C, 2):
                image_pair(i0)

        if rounds == 1:
            body()
        else:
            with tc.For_i(0, rounds, 1):
                body()


def _emit_stencil2(nc, tc, x_d, mht_d, out_d, taps, rounds):
    """Like stencil, but the matmul rhs stream is column-permuted (all even
    w's, then all odd w's) so tmp lands deinterleaved in PSUM and every
    stage-2 stencil operand is unit-stride (DVE 2x fp32 mode)."""
    h0, h1, h2, h3 = taps
    s2ops = int(os.environ.get("DWT_S2OPS", "4"))
    nofix = os.environ.get("DWT_NOFIX", "0") == "1"
    with (
        tc.tile_pool(name="consts", bufs=1) as cpool,
        tc.tile_pool(name="img", bufs=3) as img_pool,
        tc.tile_pool(name="tmp", bufs=3) as tmp_pool,
        tc.tile_pool(name="outp", bufs=3) as out_pool,
        tc.tile_pool(name="ps1", bufs=4, space=bass.MemorySpace.PSUM) as ps1_pool,
    ):
        mht_t = cpool.tile([P, KH, NQ], F32)
        nc.sync.dma_start(mht_t[:], mht_d.rearrange("(k p) q -> p k q", p=P))

        zb = [cpool.tile([P, 2, NQ], F32, tag=f"zb{z}", name=f"zb{z}")
              for z in range(2)]
        for z in range(2):
            nc.gpsimd.memset(zb[z][:], 0.0)

        def one_image(i):
            img_t = img_pool.tile([P, KH, W], F32, tag="img")
            nc.sync.dma_start(img_t[:], x_d[i].rearrange("(k p) w -> p k w", p=P))
            zb_i = zb[i % 2]
            if not nofix:
                # deinterleaved x[256] row into partition 127
                nc.sync.dma_start(
                    zb_i[127:128, :, :],
                    x_d[i, 256:257, :].rearrange("r (w t) -> r t w", t=2))
                xr = img_pool.tile([1, 2, NQ], F32, tag="xr")
                nc.sync.dma_start(
                    xr[:], x_d[i, 255:256, :].rearrange("r (w t) -> r t w", t=2))

            out_t = out_pool.tile([P, 2, NQ], F32, tag="out")
            for m in range(2):
                ps = ps1_pool.tile([P, 2, NQ], F32, tag="ps1")
                for j in range(2):
                    k = 2 * m + j
                    lhsT = mht_t[:, k, m * P:(m + 1) * P]
                    rhs = img_t[:, k, :].rearrange("p (w t) -> p t w", t=2)
                    nc.tensor.matmul(ps[:], lhsT, rhs,
                                     start=(j == 0), stop=(j == 1))
                tmp_m = tmp_pool.tile([P, 2, NQ], F32, tag="tmp")
                nc.scalar.copy(tmp_m[:], ps[:])
                if not nofix:
                    if m == 0:
                        # tmp[127] += h3 * x[256]
                        nc.vector.scalar_tensor_tensor(
                            tmp_m[:], zb_i[:], float(h3), tmp_m[:], MULT, ADD)
                    else:
                        # tmp[128] += h0 * x[255]
                        nc.vector.scalar_tensor_tensor(
                            tmp_m[0:1, :, :], xr[:], float(h0),
                            tmp_m[0:1, :, :], MULT, ADD)

                # stage 2, all unit-stride: ev = tmp_m[:,0,:], od = tmp_m[:,1,:]
                acc = out_t[:, m, :]
                nc.vector.tensor_scalar_mul(acc, tmp_m[:, 0, :], float(h1))
                if s2ops > 1:
                    nc.vector.scalar_tensor_tensor(
                        acc, tmp_m[:, 1, :], float(h2), acc, MULT, ADD)
                    nc.vector.scalar_tensor_tensor(
                        acc[:, 1:], tmp_m[:, 1, 0:255], float(h0),
                        acc[:, 1:], MULT, ADD)
                    nc.vector.scalar_tensor_tensor(
                        acc[:, 0:255], tmp_m[:, 0, 1:256], float(h3),
                        acc[:, 0:255], MULT, ADD)
            nc.sync.dma_start(out_d[i].rearrange("(m p) q -> p m q", p=P), out_t[:])

        def body():
            for i in range(C):
                one_image(i)

        if rounds == 1:
            body()
        else:
            with tc.For_i(0, rounds, 1):
                body()


def _emit_stencil(nc, tc, x_d, mht_d, out_d, taps, rounds):
    """stencil mode: 4 banded matmuls + DVE stencil per image."""
    h0, h1, h2, h3 = taps
    s2ops = int(os.environ.get("DWT_S2OPS", "4"))
    nofix = os.environ.get("DWT_NOFIX", "0") == "1"
    copy_eng = os.environ.get("DWT_COPY", "scalar")
    with (
        tc.tile_pool(name="consts", bufs=1) as cpool,
        tc.tile_pool(name="img", bufs=3) as img_pool,
        tc.tile_pool(name="tmp", bufs=3) as tmp_pool,
        tc.tile_pool(name="outp", bufs=3) as out_pool,
        tc.tile_pool(name="ps1", bufs=4, space=bass.MemorySpace.PSUM) as ps1_pool,
    ):
        # weights: mht chunk k rows, p-columns of tile m -> [128, 128] blocks
        # W[m][j] = mht[128*(2m+j) : ..+128, 128m : 128m+128], j in {0, 1}
        mht_t = cpool.tile([P, KH, NQ], F32)
        nc.sync.dma_start(mht_t[:], mht_d.rearrange("(k p) q -> p k q", p=P))

        # Engine APs need 32-aligned partition bases and TensorScalarPtr needs
        # equal base partitions on its SBUF operands, so the p=127 corner
        # (+= h3 * x[256]) goes through zero-padded [P, W] scratch tiles:
        # x[256] lands in partition 127 via DMA (no alignment limits there);
        # partitions 0..126 stay zero so the fused multiply-add over the whole
        # tmp tile only changes partition 127. Two tiles, used alternately, so
        # consecutive images don't serialize on one buffer.
        zb = [cpool.tile([P, W], F32, tag=f"zb{z}", name=f"zb{z}")
              for z in range(2)]
        for z in range(2):
            nc.gpsimd.memset(zb[z][:], 0.0)

        def one_image(i):
            img_t = img_pool.tile([P, KH, W], F32, tag="img")
            nc.sync.dma_start(img_t[:], x_d[i].rearrange("(k p) w -> p k w", p=P))
            zb_i = zb[i % 2]
            if not nofix:
                nc.sync.dma_start(zb_i[127:128, :], x_d[i, 256:257, :])
                # x[255] for the p=128 corner (partition 0 of tile 1: aligned)
                xr = img_pool.tile([1, W], F32, tag="xr")
                nc.sync.dma_start(xr[:], x_d[i, 255:256, :])

            out_t = out_pool.tile([P, 2, NQ], F32, tag="out")
            for m in range(2):
                # ---- stage 1: tmp[128m:128m+128, :] ----
                ps = ps1_pool.tile([P, W], F32, tag="ps1")
                for j in range(2):
                    k = 2 * m + j
                    lhsT = mht_t[:, k, m * P:(m + 1) * P]
                    nc.tensor.matmul(ps[:], lhsT, img_t[:, k, :],
                                     start=(j == 0), stop=(j == 1))
                tmp_m = tmp_pool.tile([P, W], F32, tag="tmp")
                if copy_eng == "vector":
                    nc.vector.tensor_copy(tmp_m[:], ps[:])
                else:
                    nc.scalar.copy(tmp_m[:], ps[:])
                # band corner fixups:
                if not nofix:
                    if m == 0:
                        # tmp[127] += h3 * x[256]
                        nc.vector.scalar_tensor_tensor(
                            tmp_m[:], zb_i[:], float(h3),
                            tmp_m[:], MULT, ADD)
                    else:
                        # tmp[128] += h0 * x[255]
                        nc.vector.scalar_tensor_tensor(
                            tmp_m[0:1, :], xr[:], float(h0),
                            tmp_m[0:1, :], MULT, ADD)

                # ---- stage 2: out rows 128m..128m+127 ----
                acc = out_t[:, m, :]
                # q=0..255: out[q] = h0*tmp[2q-1] + h1*tmp[2q] + h2*tmp[2q+1]
                #                  + h3*tmp[2q+2]   (h0@q=0, h3@q=255 clipped)
                nc.vector.tensor_scalar_mul(acc, _even(tmp_m[:]), float(h1))
                if s2ops > 1:
                    nc.vector.scalar_tensor_tensor(
                        acc, _odd(tmp_m[:]), float(h2), acc, MULT, ADD)
                    nc.vector.scalar_tensor_tensor(
                        acc[:, 1:], _even(tmp_m[:, 1:511]), float(h0),
                        acc[:, 1:], MULT, ADD)
                    nc.vector.scalar_tensor_tensor(
                        acc[:, 0:255], _even(tmp_m[:, 2:512]), float(h3),
                        acc[:, 0:255], MULT, ADD)
            nc.sync.dma_start(out_d[i].rearrange("(m p) q -> p m q", p=P), out_t[:])

        def body():
            for i in range(C):
                one_image(i)

        if rounds == 1:
            body()
        else:
            with tc.For_i(0, rounds, 1):
                body()


def _emit_matmul(nc, tc, x_d, mht_d, out_d, mode, rounds):
    """all-matmul scheme X: per-image tensor stationary, mht moving."""
    mm_dt = F32R if mode == "f32r" else F32
    windowed = mode == "f32w"

    def mmcast(ap):
        return ap.bitcast(mm_dt) if mm_dt is not F32 else ap

    with (
        tc.tile_pool(name="consts", bufs=1) as cpool,
        tc.tile_pool(name="img", bufs=3) as img_pool,
        tc.tile_pool(name="tmpT", bufs=2) as tmp_pool,
        tc.tile_pool(name="outp", bufs=3) as out_pool,
        tc.tile_pool(name="ps1", bufs=4, space=bass.MemorySpace.PSUM) as ps1_pool,
        tc.tile_pool(name="ps2", bufs=2, space=bass.MemorySpace.PSUM) as ps2_pool,
    ):
        mht_t = cpool.tile([P, KH, NQ], F32)
        nc.sync.dma_start(mht_t[:], mht_d.rearrange("(k p) q -> p k q", p=P))

        def one_image(i):
            img_t = img_pool.tile([P, KH, W], F32, tag="img")
            nc.sync.dma_start(img_t[:], x_d[i].rearrange("(k p) w -> p k w", p=P))

            # stage 1: tmpT[w, p], w-tiles m = 0..3, two per PSUM bank
            tmpT_t = tmp_pool.tile([P, KH, NQ], F32, tag="tmpT")
            for half in range(2):
                ps = ps1_pool.tile([P, 2, NQ], F32, tag="ps1")
                for ml in range(2):
                    m = 2 * half + ml
                    for k in range(KH):
                        lhsT = img_t[:, k, m * P:(m + 1) * P]
                        if windowed:
                            lo, hi = _window(k)
                            rhs = mht_t[:, k, lo:hi]
                            dst = ps[:, ml, lo:hi]
                        else:
                            rhs = mht_t[:, k, :]
                            dst = ps[:, ml, :]
                        nc.tensor.matmul(
                            dst, mmcast(lhsT), mmcast(rhs),
                            start=(k == 0), stop=(k == KH - 1),
                        )
                nc.vector.tensor_copy(tmpT_t[:, 2 * half:2 * half + 2, :], ps[:])

            # stage 2: out[p, q], p-tiles mm = 0..1, both in one PSUM bank
            out_t = out_pool.tile([P, 2, NQ], F32, tag="out")
            ps2 = ps2_pool.tile([P, 2, NQ], F32, tag="ps2")
            for mm in range(2):
                for k in range(KH):
                    lhsT = tmpT_t[:, k, mm * P:(mm + 1) * P]
                    if windowed:
                        lo, hi = _window(k)
                        rhs = mht_t[:, k, lo:hi]
                        dst = ps2[:, mm, lo:hi]
                    else:
                        rhs = mht_t[:, k, :]
                        dst = ps2[:, mm, :]
                    nc.tensor.matmul(
                        dst, mmcast(lhsT), mmcast(rhs),
                        start=(k == 0), stop=(k == KH - 1),
                    )
            nc.scalar.copy(out_t[:], ps2[:])
            nc.sync.dma_start(out_d[i].rearrange("(m p) q -> p m q", p=P), out_t[:])

        def body():
            for i in range(C):
                one_image(i)

        if rounds == 1:
            body()
        else:
            with tc.For_i(0, rounds, 1):
                body()


def build_nc(mode=MODE, rounds=1, taps=(0.0, 0.0, 0.0, 0.0)):
    nc = bacc.Bacc("TRN2", target_bir_lowering=False, debug=False,
                   num_devices=N_CORES)
    x_d = nc.dram_tensor("x", [C, H, W], F32, kind="ExternalInput").ap()
    mht_d = nc.dram_tensor("mht", [W, NQ], F32, kind="ExternalInput").ap()
    out_d = nc.dram_tensor("out", [C, 256, NQ], F32, kind="ExternalOutput").ap()
    with tile.TileContext(nc) as tc:
        if mode == "stencil":
            _emit_stencil(nc, tc, x_d, mht_d, out_d, taps, rounds)
        elif mode == "stencil2":
            _emit_stencil2(nc, tc, x_d, mht_d, out_d, taps, rounds)
        elif mode == "stencil3":
            _emit_stencil3(nc, tc, x_d, mht_d, out_d, taps, rounds)
        elif mode == "stencil4":
            _emit_stencil4(nc, tc, x_d, mht_d, out_d, taps, rounds)
        elif mode == "stencil5":
            _emit_stencil5(nc, tc, x_d, mht_d, out_d, taps, rounds)
        elif mode == "stencil6":
            _emit_stencil6(nc, tc, x_d, mht_d, out_d, taps, rounds)
        elif mode == "stencil7":
            _emit_stencil7(nc, tc, x_d, mht_d, out_d, taps, rounds)
        else:
            _emit_matmul(nc, tc, x_d, mht_d, out_d, mode, rounds)
    nc.compile()
    return nc


def extract_taps(mht: np.ndarray) -> tuple:
    """Band taps from the actual filter matrix: mht[2r-1+t, r] = h_t.
    Row r=1 keeps all four taps in-range."""
    return tuple(float(mht[1 + t, 1]) for t in range(4))


_NC_CACHE = {}


def get_nc(mode=MODE, rounds=1, taps=(0.0, 0.0, 0.0, 0.0)):
    key = (mode, rounds, taps if mode == "stencil" else None)
    if key not in _NC_CACHE:
        _NC_CACHE[key] = build_nc(mode, rounds=rounds, taps=taps)
    return _NC_CACHE[key]


def kernel(input, matrix_h, matrix_h_t):
    input = np.asarray(input, dtype=np.float32)
    mht = np.ascontiguousarray(np.asarray(matrix_h_t, dtype=np.float32))
    nc = get_nc(MODE, 1, extract_taps(mht))
    in_maps = [
        {"x": np.ascontiguousarray(input[b]), "mht": mht} for b in range(N_CORES)
    ]
    res = run_bass_kernel_spmd(nc, in_maps, core_ids=list(range(N_CORES)))
    return np.stack([res.results[b]["out"] for b in range(N_CORES)], axis=0)
